# revision 3
# baseline (speedup 1.0000x reference)
import sys
if '/opt/trn_rl_repo' not in sys.path:
    sys.path.insert(0, '/opt/trn_rl_repo')
import numpy as np
import ml_dtypes

bfloat16 = ml_dtypes.bfloat16

P = 128
D = 64
E2 = 2 * D          # padded row width of gather tables (256B bf16)
WG = 8              # windows per PSUM-bank group
SG = 3              # groups per supergroup (gather-call batching unit)
NC_ = 8
GK = 48             # max tiles per dma_gather call

N_USER, N_REST = 200000, 50000
CHUNK = 1024                                        # transform chunk rows
_QCTR = [0]


def _q():
    q = _QCTR[0] % 4
    _QCTR[0] += 1
    return q


def _derive():
    global USLICE, RSLICE, WB, WA, UPAD, RPAD, WBR, WAR
    USLICE, RSLICE = N_USER // NC_, N_REST // NC_
    WB = -(-USLICE // P); WB = -(-WB // WG) * WG
    WA = -(-RSLICE // P); WA = -(-WA // WG) * WG
    UPAD, RPAD = WB * P, WA * P
    WBR = -(-USLICE // P)
    WAR = -(-RSLICE // P)


_derive()


def _wrap16(idx):
    n = len(idx)
    assert n % 16 == 0
    return np.tile(idx.reshape(-1, 16).T, (8, 1)).astype(np.int16)


def _build_side(src, dst, n_w, dst_slice, src_slice, src_pad, nbk):
    """Tapes + schedule for one side. nbk = gather buckets (8 for user-table
    sides, 2 for rest-table sides). Bucket of owner o = o // (8//nbk);
    local idx = (o % (8//nbk)) * src_pad + (src % src_slice).

    Tile stream (gather order): for sg, for bucket, for group in sg, for
    window, tiles. dstrel tape is window-major instead. Returns python
    schedule for emission.
    """
    opb = NC_ // nbk                      # owners per bucket
    owner_d = dst // dst_slice
    per_core = []
    n_g = n_w // WG
    sgs = [list(range(s, min(s + SG, n_g))) for s in range(0, n_g, SG)]
    counts = np.zeros((NC_, n_w, nbk), np.int64)
    for c in range(NC_):
        m = owner_d == c
        s_, d_ = src[m], dst[m]
        dloc = d_ - c * dst_slice
        w = dloc // P
        so = s_ // src_slice
        b = so // opb
        sl = (so % opb) * src_pad + (s_ - so * src_slice)
        order = np.argsort((w * nbk + b) * (dst_slice + P) + dloc, kind='stable')
        np.add.at(counts[c], (w, b), 1)
        per_core.append((sl[order], dloc[order], w[order], b[order]))
    tc_wb = -(-counts.max(axis=0) // P)               # [n_w, nbk]
    tcw = tc_wb.sum(axis=1)                           # tiles per window
    total_tiles = int(tcw.sum())

    # stream position of each (w, b) segment in gather order
    pos_wb = {}
    ti = 0
    calls = []          # (bucket, tile_lo, tile_hi) in stream order
    for sg in sgs:
        for b in range(nbk):
            lo = ti
            for g in sg:
                for w in range(g * WG, (g + 1) * WG):
                    if tc_wb[w, b] > 0:
                        pos_wb[(w, b)] = ti
                        ti += int(tc_wb[w, b])
            while lo < ti:
                calls.append((b, lo, min(lo + GK, ti)))
                lo = min(lo + GK, ti)
    assert ti == total_tiles
    # window-major dstrel order: window w's tiles = its (w,b) segs in b order
    dpos_w = {}
    dp = 0
    for w in range(n_w):
        dpos_w[w] = dp
        dp += int(tcw[w])
    # tile j of window w (0..tcw[w]) -> gather stream index
    def wtile_stream(w):
        out = []
        for b in range(nbk):
            t0 = pos_wb.get((w, b))
            if t0 is not None:
                out.extend(range(t0, t0 + int(tc_wb[w, b])))
        return out
    stream_of_w = {w: wtile_stream(w) for w in range(n_w)}

    tapes = []
    for c in range(NC_):
        sl, dloc, w, b = per_core[c]
        idxs = np.zeros(total_tiles * P, np.int64)
        drel = np.full(total_tiles * P, -1.0, np.float32)
        for w_ in range(n_w):
            for b_ in range(nbk):
                tcb = int(tc_wb[w_, b_])
                if tcb == 0:
                    continue
                m = (w == w_) & (b == b_)
                ss, dd = sl[m], dloc[m]
                n = len(ss)
                t0 = pos_wb[(w_, b_)]
                idxs[t0 * P:t0 * P + n] = ss
                # dstrel goes at window-major position
                srt = stream_of_w[w_]
                # offset of this segment within window stream
                off = srt.index(t0)
                d0 = (dpos_w[w_] + off) * P
                drel[d0:d0 + n] = (dd - w_ * P).astype(np.float32)
        tapes.append((_wrap16(idxs),
                      drel.reshape(total_tiles, P).T.astype(bfloat16).copy()))
    return dict(total_tiles=total_tiles, calls=calls, sgs=sgs, nbk=nbk,
                tc_wb=tc_wb, tcw=tcw, dpos_w=dpos_w, stream_of_w=stream_of_w,
                tapes=tapes, counts=counts)


def _build_decoder(row, col):
    owner = row // USLICE
    per_core = [np.nonzero(owner == c)[0] for c in range(NC_)]
    # zr bucketed by col-owner//4 (2 buckets); order edges by bucket
    ntl = []
    ords = []
    for c in range(NC_):
        m = per_core[c]
        cb = col[m] // (4 * RSLICE)
        o = np.argsort(cb, kind='stable')
        m = m[o]
        ords.append(m)
        nb0 = int((cb == 0).sum())
        ntl.append((-(-nb0 // P), -(-(len(m) - nb0) // P)))
    nt0 = max(t[0] for t in ntl)
    nt1 = max(t[1] for t in ntl)
    n_tiles = nt0 + nt1
    tapes = []
    for c in range(NC_):
        m = ords[c]
        cb = col[m] // (4 * RSLICE)
        zu = np.zeros(n_tiles * P, np.int64)
        zr = np.zeros(n_tiles * P, np.int64)
        lab = np.full(n_tiles * P, -1, np.int64)
        n0 = int((cb == 0).sum())
        m0, m1 = m[:n0], m[n0:]
        for (mm, base) in ((m0, 0), (m1, nt0 * P)):
            zu[base:base + len(mm)] = row[mm] % USLICE
            co = col[mm] // RSLICE
            zr[base:base + len(mm)] = (co % 4) * RPAD + (col[mm] - co * RSLICE)
            lab[base:base + len(mm)] = mm
        tapes.append((_wrap16(zu), _wrap16(zr), lab))
    return dict(n_tiles=n_tiles, nt0=nt0, nt1=nt1, tapes=tapes)


def _emit_side(nc, bass, mybir, pools, info, tn, side, layer):
    f32 = mybir.dt.float32
    bf16 = mybir.dt.bfloat16
    (gpool, ohpool, evpool, pspool) = pools
    n_w = WA if side == 'A' else WB
    n_real = WAR if side == 'A' else WBR
    tbl = tn[f'tW{layer}{"u" if side == "A" else "r"}_AG']
    span = (UPAD if side == 'A' else 4 * RPAD)
    idxt = tn[f'idx_{side}']
    dstrel = tn[f'dstrel_{side}']
    recip = tn[f'recip_{side}']
    iota = tn['iota']
    rtbl = tn[f'r{layer}{"r" if side == "A" else "u"}']
    out_tbl = tn[('h' if layer == 1 else 'z') + ('r' if side == 'A' else 'u')]
    relu = layer == 1
    tc_wb, tcw = info['tc_wb'], info['tcw']
    nbk = info['nbk']

    # gather calls (stream order); remember tile -> (xg tile, slot)
    tile_slot = {}
    for (b, lo, hi) in info['calls']:
        k = hi - lo
        xg = gpool.tile([P, GK * E2], bf16, tag="xg", name=f"xg{side}{layer}_{lo}")
        nc.gpsimd.dma_gather(
            xg[:, 0:k * E2].rearrange("p (k e) -> p k e", e=E2),
            tbl[b * span:(b + 1) * span, :],
            idxt[:, lo * 8:hi * 8],
            k * P, k * P, E2, single_packet=False)
        for t in range(k):
            tile_slot[lo + t] = (xg, t)

    for sg in info['sgs']:
        banks = {}
        for g in sg:
            if g * WG >= n_real:
                continue
            bank = pspool.tile([P, WG * D], f32, tag="bank",
                               name=f"bk{side}{layer}_{g}")
            banks[g] = bank
            for w_ in range(g * WG, (g + 1) * WG):
                tcw_ = int(tcw[w_])
                if tcw_ == 0 or w_ >= n_real:
                    continue
                d0 = info['dpos_w'][w_]
                oh = ohpool.tile([P, int(tcw.max()) * P], bf16, tag="oh",
                                 name=f"oh{side}{layer}_{w_}")
                nc.vector.tensor_tensor(
                    out=oh[:, 0:tcw_ * P].rearrange("p (k q) -> p k q", q=P),
                    in0=dstrel[:, d0:d0 + tcw_]
                        .rearrange("p (k one) -> p k one", one=1)
                        .to_broadcast([P, tcw_, P]),
                    in1=iota[:].rearrange("p (one q) -> p one q", one=1)
                        .to_broadcast([P, tcw_, P]),
                    op=mybir.AluOpType.is_equal,
                )
                stream = info['stream_of_w'][w_]
                for t, sj in enumerate(stream):
                    xg, slot = tile_slot[sj]
                    nc.tensor.matmul(
                        bank[:, (w_ - g * WG) * D:(w_ - g * WG + 1) * D],
                        lhsT=oh[:, t * P:(t + 1) * P],
                        rhs=xg[:, slot * E2:slot * E2 + D],
                        start=(t == 0), stop=(t == len(stream) - 1),
                    )
        # evict supergroup
        for g in sg:
            if g not in banks:
                continue
            wlo = g * WG
            whi = min((g + 1) * WG, n_real)
            rt = evpool.tile([P, WG * D], bf16, tag="rt",
                             name=f"rt{side}{layer}_{g}")
            nc.sync.dma_start(rt[:], rtbl[:, wlo * D:(wlo + WG) * D])
            hrow = evpool.tile([P, WG * E2], bf16, tag="hrow",
                               name=f"hr{side}{layer}_{g}")
            nc.vector.memset(hrow[:], 0.0)
            nc.vector.memset(
                hrow[:].rearrange("p (w e) -> p w e", e=E2)[:, :, D:D + 1], 1.0)
            for w_ in range(wlo, whi):
                cs = slice((w_ - wlo) * D, (w_ - wlo + 1) * D)
                ocs = slice((w_ - wlo) * E2, (w_ - wlo) * E2 + D)
                if int(tcw[w_]) > 0:
                    ms = evpool.tile([P, D], f32, tag="ms",
                                     name=f"ms{side}{layer}_{w_}")
                    nc.vector.tensor_scalar(
                        out=ms[:], in0=banks[g][:, cs],
                        scalar1=recip[:, w_:w_ + 1], scalar2=None,
                        op0=mybir.AluOpType.mult)
                    if relu:
                        m2 = evpool.tile([P, D], f32, tag="m2",
                                         name=f"m2{side}{layer}_{w_}")
                        nc.vector.tensor_tensor(out=m2[:], in0=ms[:],
                                                in1=rt[:, cs],
                                                op=mybir.AluOpType.add)
                        nc.vector.tensor_scalar(
                            out=hrow[:, ocs], in0=m2[:], scalar1=0.0,
                            scalar2=None, op0=mybir.AluOpType.max)
                    else:
                        nc.vector.tensor_tensor(out=hrow[:, ocs], in0=ms[:],
                                                in1=rt[:, cs],
                                                op=mybir.AluOpType.add)
                else:
                    if relu:
                        nc.vector.tensor_scalar(
                            out=hrow[:, ocs], in0=rt[:, cs], scalar1=0.0,
                            scalar2=None, op0=mybir.AluOpType.max)
                    else:
                        nc.vector.tensor_copy(out=hrow[:, ocs], in_=rt[:, cs])
            nc.sync.dma_start(
                out_tbl[wlo * P:(wlo + WG) * P, :]
                .rearrange("(w p) e -> p w e", p=P),
                hrow[:].rearrange("p (w e) -> p w e", e=E2))


def _emit_transform(nc, mybir, pools, tn, src_kind, n_rows, outs, name):
    """Stream a row table through PE: out_tile = x_tile @ W (+ b via ones row).

    src_kind ('host', sbuf_const_name [65, n_rows]) or ('dram', row_tbl_name
    [n_rows, E2-padded or D]): loaded transposed via dma_start_transpose.
    outs: (Wstack_name, use_ones, 'rows16'|'cols', dest) — rows16: padded
    [n_rows, E2] bf16 gather table; cols: window-major [P, (n_rows/P)*D].
    """
    f32 = mybir.dt.float32
    bf16 = mybir.dt.bfloat16
    (tpool, evpool, pspool) = pools
    for c0 in range(0, n_rows, CHUNK):
        rows = min(CHUNK, n_rows - c0)
        kt = rows // P
        if src_kind[0] == 'hostdram':
            xc = tpool.tile([D + 1, CHUNK], bf16, tag="xc", name=f"xc{name}_{c0}")
            nc.sync.dma_start(xc[:, 0:rows], tn[src_kind[1]][:, c0:c0 + rows])
            lhs_all = xc
        else:
            ct = tpool.tile([P, CHUNK], bf16, tag="ct", name=f"ct{name}_{c0}")
            nc.sync.dma_start_transpose(
                ct[:, 0:rows], tn[src_kind[1]][c0:c0 + rows, :])
            lhs_all = ct
        bufs = {}
        for oi, (wname, use_ones, dkind, dest) in enumerate(outs):
            cw = E2 if dkind == 'rows16' else D
            ob = evpool.tile([P, CHUNK // P * cw], bf16, tag=f"ob{oi}",
                             name=f"ob{name}_{oi}_{c0}")
            if dkind == 'rows16':
                nc.vector.memset(ob[:], 0.0)
            bufs[oi] = ob
        for j in range(kt):
            for oi, (wname, use_ones, dkind, dest) in enumerate(outs):
                kdim = D + 1 if use_ones else D
                cw = E2 if dkind == 'rows16' else D
                ps = pspool.tile([P, D], f32, tag="tps",
                                 name=f"tp{name}_{oi}_{c0}_{j}")
                nc.tensor.matmul(
                    ps[:], lhsT=lhs_all[0:kdim, j * P:(j + 1) * P],
                    rhs=tn[wname][0:kdim, :], start=True, stop=True)
                nc.vector.tensor_copy(out=bufs[oi][:, j * cw:j * cw + D],
                                      in_=ps[:])
        for oi, (wname, use_ones, dkind, dest) in enumerate(outs):
            if dkind == 'rows16':
                nc.sync.dma_start(
                    tn[dest][c0:c0 + rows, :]
                    .rearrange("(w p) e -> p w e", p=P),
                    bufs[oi][:, 0:kt * E2]
                    .rearrange("p (w e) -> p w e", e=E2))
            else:
                nc.sync.dma_start(
                    tn[dest][:, (c0 // P) * D:(c0 // P + kt) * D],
                    bufs[oi][:, 0:kt * D])


def run(inputs):
    from concourse import bass, mybir, bacc, tile
    from concourse.bass_utils import run_bass_kernel_spmd
    import os

    f32 = mybir.dt.float32
    bf16 = mybir.dt.bfloat16
    i16 = mybir.dt.int16

    e_u2r = np.asarray(inputs['edge_u2r']).astype(np.int64)
    e_r2u = np.asarray(inputs['edge_r2u']).astype(np.int64)
    eli = np.asarray(inputs['edge_label_index']).astype(np.int64)

    # side A: dst=rest, gathers user tables (8 buckets of UPAD)
    # side B: dst=user, gathers rest tables (2 buckets of 4*RPAD)
    infoA = _build_side(e_u2r[0], e_u2r[1], WA, RSLICE, USLICE, UPAD, 8)
    infoB = _build_side(e_r2u[0], e_r2u[1], WB, USLICE, RSLICE, RPAD, 2)
    infoD = _build_decoder(eli[0], eli[1])

    cntR = np.bincount(e_u2r[1], minlength=N_REST).astype(np.float32)
    cntU = np.bincount(e_r2u[1], minlength=N_USER).astype(np.float32)
    recipR = 1.0 / np.maximum(cntR, 1.0)
    recipU = 1.0 / np.maximum(cntU, 1.0)

    x_user = np.asarray(inputs['x_user'], np.float32)
    x_rest = np.asarray(inputs['x_rest'], np.float32)

    _QCTR[0] = 0
    nc = bacc.Bacc("TRN2", target_bir_lowering=False, debug=False,
                   num_devices=NC_, num_swdge_queues=4)
    T = {}
    T['xTu_d'] = nc.dram_tensor("xTu_d", [D + 1, UPAD], bf16, kind="ExternalInput")
    T['xTr_d'] = nc.dram_tensor("xTr_d", [D + 1, RPAD], bf16, kind="ExternalInput")
    wnames = ['W1l_u2r_s', 'W1rb_u2r_s', 'W1l_r2u_s', 'W1rb_r2u_s',
              'W2l_u2r_s', 'W2rb_u2r_s', 'W2l_r2u_s', 'W2rb_r2u_s',
              'Wub_dec_s', 'Wrb_dec_s']
    for nm in wnames:
        T[nm] = nc.dram_tensor(nm, [D + 1, D], bf16, kind="ExternalInput")
    T['iota_d'] = nc.dram_tensor("iota_d", [P, P], bf16, kind="ExternalInput")
    T['recipA_d'] = nc.dram_tensor("recipA_d", [P, WA], f32, kind="ExternalInput")
    T['recipB_d'] = nc.dram_tensor("recipB_d", [P, WB], f32, kind="ExternalInput")
    for sd, inf in (('A', infoA), ('B', infoB)):
        T[f'idx_{sd}_d'] = nc.dram_tensor(
            f"idx_{sd}_d", [P, inf['total_tiles'] * 8], i16, kind="ExternalInput")
        T[f'dstrel_{sd}_d'] = nc.dram_tensor(
            f"dstrel_{sd}_d", [P, inf['total_tiles']], bf16, kind="ExternalInput")
    T['zu16_d'] = nc.dram_tensor("zu16_d", [P, infoD['n_tiles'] * 8], i16,
                                 kind="ExternalInput")
    T['zr16_d'] = nc.dram_tensor("zr16_d", [P, infoD['n_tiles'] * 8], i16,
                                 kind="ExternalInput")
    dec_out = nc.dram_tensor("dec_out", [P, infoD['n_tiles']], f32,
                             kind="ExternalOutput")

    with tile.TileContext(nc) as tc:
        with tc.tile_pool(name="consts", bufs=1) as consts, \
             tc.tile_pool(name="gpool", bufs=5) as gpool, \
             tc.tile_pool(name="ohpool", bufs=2) as ohpool, \
             tc.tile_pool(name="evpool", bufs=2) as evpool, \
             tc.tile_pool(name="obpool", bufs=2) as obpool, \
             tc.tile_pool(name="tpool", bufs=2) as tpool, \
             tc.tile_pool(name="pspool", bufs=6, space="PSUM") as pspool, \
             tc.tile_pool(name="tpsum", bufs=2, space="PSUM") as tpsum, \
             tc.tile_pool(name="dram", bufs=1, space="DRAM") as dram:

            tn = dict(T)

            def ld(name, dname, shape, dt):
                t = consts.tile(shape, dt, name=name)
                nc.sync.dma_start(t[:], T[dname][:])
                tn[name] = t
            for nm in wnames:
                ld(nm[:-2], nm, [D + 1, D], bf16)
            ld('iota', 'iota_d', [P, P], bf16)
            ld('recip_A', 'recipA_d', [P, WA], f32)
            ld('recip_B', 'recipB_d', [P, WB], f32)
            for sd, inf in (('A', infoA), ('B', infoB)):
                ld(f'idx_{sd}', f'idx_{sd}_d', [P, inf['total_tiles'] * 8], i16)
                ld(f'dstrel_{sd}', f'dstrel_{sd}_d', [P, inf['total_tiles']], bf16)
            ld('zu16', 'zu16_d', [P, infoD['n_tiles'] * 8], i16)
            ld('zr16', 'zr16_d', [P, infoD['n_tiles'] * 8], i16)

            def dr(name, shape, shared=False):
                tn[name] = dram.tile(shape, bf16, name=name,
                                     **(dict(addr_space='Shared') if shared else {}))
            dr('tW1u_in', [UPAD, E2]); dr('tW1u_AG', [NC_ * UPAD, E2], True)
            dr('tW1r_in', [RPAD, E2]); dr('tW1r_AG', [NC_ * RPAD, E2], True)
            dr('tW2u_in', [UPAD, E2]); dr('tW2u_AG', [NC_ * UPAD, E2], True)
            dr('tW2r_in', [RPAD, E2]); dr('tW2r_AG', [NC_ * RPAD, E2], True)
            dr('zWr_in', [RPAD, E2]);  dr('zWr_AG', [NC_ * RPAD, E2], True)
            dr('zWu', [UPAD, E2])
            dr('hu', [UPAD, E2]); dr('hr', [RPAD, E2])
            dr('zu', [UPAD, E2]); dr('zr', [RPAD, E2])
            dr('r1u', [P, WB * D]); dr('r1r', [P, WA * D])
            dr('r2u', [P, WB * D]); dr('r2r', [P, WA * D])

            tpools = (tpool, obpool, tpsum)
            lpools = (gpool, ohpool, evpool, pspool)

            def AG(inn, outn):
                nc.gpsimd.collective_compute(
                    "AllGather", mybir.AluOpType.bypass,
                    replica_groups=[list(range(NC_))],
                    ins=[tn[inn].opt()], outs=[tn[outn].opt()])

            _emit_transform(nc, mybir, tpools, tn, ('hostdram', 'xTr_d'), RPAD,
                            [('W1l_r2u', False, 'rows16', 'tW1r_in'),
                             ('W1rb_u2r', True, 'cols', 'r1r')], 't1r')
            AG('tW1r_in', 'tW1r_AG')
            _emit_transform(nc, mybir, tpools, tn, ('hostdram', 'xTu_d'), UPAD,
                            [('W1l_u2r', False, 'rows16', 'tW1u_in'),
                             ('W1rb_r2u', True, 'cols', 'r1u')], 't1u')
            AG('tW1u_in', 'tW1u_AG')
            _emit_side(nc, bass, mybir, lpools, infoB, tn, 'B', 1)
            _emit_side(nc, bass, mybir, lpools, infoA, tn, 'A', 1)
            _emit_transform(nc, mybir, tpools, tn, ('dram', 'hr'), RPAD,
                            [('W2l_r2u', False, 'rows16', 'tW2r_in'),
                             ('W2rb_u2r', True, 'cols', 'r2r')], 't2r')
            AG('tW2r_in', 'tW2r_AG')
            _emit_transform(nc, mybir, tpools, tn, ('dram', 'hu'), UPAD,
                            [('W2l_u2r', False, 'rows16', 'tW2u_in'),
                             ('W2rb_r2u', True, 'cols', 'r2u')], 't2u')
            AG('tW2u_in', 'tW2u_AG')
            _emit_side(nc, bass, mybir, lpools, infoB, tn, 'B', 2)
            _emit_side(nc, bass, mybir, lpools, infoA, tn, 'A', 2)
            _emit_transform(nc, mybir, tpools, tn, ('dram', 'zr'), RPAD,
                            [('Wrb_dec', True, 'rows16', 'zWr_in')], 't3r')
            AG('zWr_in', 'zWr_AG')
            _emit_transform(nc, mybir, tpools, tn, ('dram', 'zu'), UPAD,
                            [('Wub_dec', True, 'rows16', 'zWu')], 't3u')

            # ---- decoder
            outsb = consts.tile([P, infoD['n_tiles']], f32, name='outsb')
            nt, nt0 = infoD['n_tiles'], infoD['nt0']
            chunks = []
            for (b, lo, hi) in ((0, 0, nt0), (1, nt0, nt)):
                for g0 in range(lo, hi, GK):
                    chunks.append((b, g0, min(GK, hi - g0)))
            dec_calls = []
            for (b, g0, k) in chunks:
                pair = []
                for (tape, tblv, bb, span) in (
                        ('zu16', tn['zWu'], 0, UPAD),
                        ('zr16', tn['zWr_AG'], b, 4 * RPAD)):
                    xg = gpool.tile([P, GK * E2], bf16, tag="xg",
                                    name=f"d{tape}_{g0}")
                    nc.gpsimd.dma_gather(
                        xg[:, 0:k * E2].rearrange("p (k e) -> p k e", e=E2),
                        tblv[bb * span:(bb + 1) * span, :],
                        tn[tape][:, g0 * 8:(g0 + k) * 8],
                        k * P, k * P, E2, single_packet=False,
                        queue_num=_q())
                    pair.append(xg)
                dec_calls.append((g0, k, pair[0], pair[1]))
            for (g0, k, zu_t, zr_t) in dec_calls:
                pr = evpool.tile([P, GK * D], bf16, tag="pr", name=f"pr{g0}")
                nc.vector.tensor_tensor(
                    out=pr[:, 0:k * D].rearrange("p (k f) -> p k f", f=D),
                    in0=zu_t[:, 0:k * E2].rearrange("p (k e) -> p k e", e=E2)[:, :, 0:D],
                    in1=zr_t[:, 0:k * E2].rearrange("p (k e) -> p k e", e=E2)[:, :, 0:D],
                    op=mybir.AluOpType.mult)
                nc.vector.reduce_sum(
                    outsb[:, g0:g0 + k].rearrange("p (k one) -> p k one", one=1),
                    pr[:, 0:k * D].rearrange("p (k f) -> p k f", f=D),
                    axis=mybir.AxisListType.X)
            nc.sync.dma_start(dec_out[:], outsb[:])

    nc.compile()

    # ---------------- host data ----------------
    def xT_slices(x, slice_n, pad):
        out = np.zeros((NC_, D + 1, pad), np.float32)
        for c in range(NC_):
            xs = x[c * slice_n:(c + 1) * slice_n]
            out[c, :D, :xs.shape[0]] = xs.T
            out[c, D, :] = 1.0
        return out.astype(bfloat16)
    xTu = xT_slices(x_user, USLICE, UPAD)
    xTr = xT_slices(x_rest, RSLICE, RPAD)

    def wstack(wl, b=None):
        w = np.zeros((D + 1, D), np.float32)
        w[:D] = np.asarray(inputs[wl], np.float32)
        if b is not None:
            w[D] = np.asarray(inputs[b], np.float32)
        return w.astype(bfloat16)

    wmaps = {
        'W1l_u2r_s': wstack('W1l_u2r'), 'W1rb_u2r_s': wstack('W1r_u2r', 'b1_u2r'),
        'W1l_r2u_s': wstack('W1l_r2u'), 'W1rb_r2u_s': wstack('W1r_r2u', 'b1_r2u'),
        'W2l_u2r_s': wstack('W2l_u2r'), 'W2rb_u2r_s': wstack('W2r_u2r', 'b2_u2r'),
        'W2l_r2u_s': wstack('W2l_r2u'), 'W2rb_r2u_s': wstack('W2r_r2u', 'b2_r2u'),
        'Wub_dec_s': wstack('Wu_dec', 'bu_dec'),
        'Wrb_dec_s': wstack('Wr_dec', 'br_dec'),
    }

    def recip_tape(recip, slice_, n_w):
        out = np.ones((NC_, P, n_w), np.float32)
        for c in range(NC_):
            r = recip[c * slice_:(c + 1) * slice_]
            pad = np.ones(n_w * P, np.float32)
            pad[:len(r)] = r
            out[c] = pad.reshape(n_w, P).T
        return out
    rA = recip_tape(recipR, RSLICE, WA)
    rB = recip_tape(recipU, USLICE, WB)
    iota_np = np.tile(np.arange(P, dtype=np.float32), (P, 1)).astype(bfloat16)

    in_maps = []
    for c in range(NC_):
        m = {
            'xTu_d': xTu[c], 'xTr_d': xTr[c],
            'iota_d': iota_np, 'recipA_d': rA[c], 'recipB_d': rB[c],
            'idx_A_d': infoA['tapes'][c][0], 'dstrel_A_d': infoA['tapes'][c][1],
            'idx_B_d': infoB['tapes'][c][0], 'dstrel_B_d': infoB['tapes'][c][1],
            'zu16_d': infoD['tapes'][c][0], 'zr16_d': infoD['tapes'][c][1],
        }
        m.update(wmaps)
        in_maps.append(m)

    import os
    if os.environ.get("KSIM") == "1":
        from concourse import bass_interp

        class _R:
            pass
        sim = bass_interp.MultiCoreSim(nc, NC_)
        for c in range(NC_):
            for k, v in in_maps[c].items():
                sim.cores[c].tensor(k)[:] = v
            sim.cores[c].tensor("dec_out")[:] = 0
        sim.simulate()
        res = _R()
        res.results = [{"dec_out": sim.cores[c].mem_tensor("dec_out").copy()}
                       for c in range(NC_)]
        res.exec_time_ns = None
    else:
        trace = os.environ.get("KTRACE", "0") == "1"
        res = run_bass_kernel_spmd(nc, in_maps, core_ids=list(range(NC_)),
                                   trace=trace)
        if trace and res.exec_time_ns:
            print(f"HW exec time: {res.exec_time_ns} ns")

    out = np.zeros(eli.shape[1], np.float32)
    for c in range(NC_):
        vals = res.results[c]["dec_out"]
        lab = infoD['tapes'][c][2]
        ntile = len(lab) // P
        toks = vals[:, :ntile].T.reshape(-1)
        valid = lab >= 0
        out[lab[valid]] = toks[valid]
    return out


def kernel(**inputs):
    return run(inputs)
